# revision 1
# baseline (speedup 1.0000x reference)
"""Trainium2 Bass kernel for nn_CombinedLoss (rec + ident + attention-BCE).

Strategy
--------
The 256 MB correspondence_matrices BCE dominates (memory-bound regime).
gt_corr is nonzero only on the 5 diagonals |i-j|<=2, so

    bce_sum = sum softplus(x)  +  sum_band [2g*softplus(x) - (g+2g^2)*x]

Main stream (per core, 8 matrices shipped as fp16 = 16 MB):
  softplus(x) = -ln(sigmoid(-x)), and only the SUM is needed, so
      sum softplus = -ln( prod sigmoid(-x) ).
  One ScalarE Sigmoid pass per element (half the ACT cost of the exp+ln
  two-pass form, no table swaps), then the Vector engine folds the bf16
  sigmoid values with a 5-level pairwise-product tree (2x-rate 16-bit
  tensor_tensor ops) into groups of 32.  The tiny product tiles
  ([128,256] per matrix) are DMA'd out; the host takes log in f64.
  Group products stay within bf16 range (~e^-53 worst case).

Band correction: strided DMA gathers the 5 diagonals into a [128,320]
fp16 tile; softplus is evaluated on DVE with an even polynomial
softplus(x) = x/2 + c0 + u*R(u), u = x^2 (deg-4 LS fit, |err|<2.5e-2,
fine for a 0.6% correction term), then two weighted reduces against
host-precomputed banded weights g1/g2.

rec + ident losses are tiny and sharded uniformly: each core takes 1/8
of the reconstruction points and 2 of the 16 (view,batch) identity
pairs, writing [128,24] f32 partials; the host combines partials and
takes the final few scalar ops in f64.

Engine budget per core (cost model): ACT ~56us (8 sigmoid passes),
DMA ~53us (16 MB fp16 stream + small inputs), DVE ~48us (product
trees + band + rec + ident).
"""

import dataclasses
import os

import numpy as np

import concourse.bacc as bacc
import concourse.bass as bass
import concourse.mybir as mybir
from concourse.bass_utils import run_bass_kernel_spmd
from concourse.tile import TileContext

F32 = mybir.dt.float32
F16 = mybir.dt.float16
BF16 = mybir.dt.bfloat16
I32 = mybir.dt.int32
AF = mybir.ActivationFunctionType
OP = mybir.AluOpType
AX = mybir.AxisListType

N = 1024
V = 4
B = 4
F_FRAMES = 16
NCORES = 8
MAT_PER_CORE = 8          # V*V*B / 8
MATSZ = N * N             # elements per matrix
CORR_LEN = 2 + MAT_PER_CORE * MATSZ + 2
NHALF = 2 * MAT_PER_CORE  # sigmoid passes per core ([128,4096] each)
HALF = MATSZ // 2
KGRP = 16                 # product-group size
PRODC = MAT_PER_CORE * MATSZ // 128 // KGRP   # 4096 product columns

# softplus(x) ~= x/2 + PC0 + u*(PC1 + PC2 u + PC3 u^2 + PC4 u^3), u = x^2
PC0 = 0.6949000596
PC1 = 0.1198186111
PC2 = -3.2243449876e-3
PC3 = 6.6297696212e-5
PC4 = -5.4602088069e-7

# rec shard: 1/8 of B*F*N = 65536 points -> 8192 points = [128, 64, 3]
REC_PTS = 8192

# final_acc column layout
C_ATT_C1 = 0
C_ATT_C2 = 1
C_REC_SE = 2
C_REC_NUM = 3
C_REC_MN = 4     # 4..6
C_REC_MX = 7     # 7..9
C_ID_ERR = 10    # 10..13 (vb0x, vb0y, vb1x, vb1y)
C_ID_MN = 14     # 14..17
C_ID_MX = 18     # 18..21
NCOLS = 24

_CACHE = {}
LAST_RESULTS = None


def _ap(t, offset, pairs):
    """Custom access pattern on a DRAM tensor handle."""
    return dataclasses.replace(t[:], ap=[list(p) for p in pairs], offset=offset)


def _build_program():
    parts = set(
        os.environ.get("KERNEL_PARTS", "main,diag,rec,ident").split(",")
    )
    nc = bacc.Bacc("TRN2", target_bir_lowering=False, debug=False)

    corr = nc.dram_tensor("corrpad", [CORR_LEN], F16, kind="ExternalInput")
    recpred = nc.dram_tensor("recpred", [128, 192], F16, kind="ExternalInput")
    recgt = nc.dram_tensor("recgt", [128, 192], F16, kind="ExternalInput")
    recvis = nc.dram_tensor("recvis", [128, 64], I32, kind="ExternalInput")
    g1c = nc.dram_tensor("g1c", [128, 320], F16, kind="ExternalInput")
    g2c = nc.dram_tensor("g2c", [128, 320], F16, kind="ExternalInput")
    trk = nc.dram_tensor("trk", [2, 128, 256], F16, kind="ExternalInput")
    iprd = nc.dram_tensor("iprd", [2, 128, 384], F32, kind="ExternalInput")
    projbc = nc.dram_tensor("projbc", [128, 24], F32, kind="ExternalInput")
    out_d = nc.dram_tensor("out", [128, NCOLS], F32, kind="ExternalOutput")
    prods_d = nc.dram_tensor("prods", [128, PRODC], BF16, kind="ExternalOutput")

    with TileContext(nc) as tc:
        with (
            tc.tile_pool(name="xpool", bufs=3) as xpool,
            tc.tile_pool(name="spool", bufs=2) as spool,
            tc.tile_pool(name="tpool", bufs=2) as tpool,
            tc.tile_pool(name="cpool", bufs=1) as cpool,
        ):
            fin = cpool.tile([128, NCOLS], F32, tag="fin")
            outp = cpool.tile([128, PRODC], BF16, tag="outp")
            nc.vector.memset(fin[:], 0.0)

            # persistent small-input tiles
            if "rec" in parts:
                prt = cpool.tile([128, 192], F16, tag="prt")
                grt = cpool.tile([128, 192], F16, tag="grt")
                vrt = cpool.tile([128, 64], I32, tag="vrt")
            if "ident" in parts:
                psb = cpool.tile([128, 24], F32, tag="psb")
                tks = [
                    cpool.tile([128, 256], F16, tag=f"tk{i}", name=f"tk{i}")
                    for i in range(2)
                ]
                pds = [
                    cpool.tile([128, 384], F32, tag=f"pd{i}", name=f"pd{i}")
                    for i in range(2)
                ]
            if "diag" in parts:
                g1t = cpool.tile([128, 320], F16, tag="g1t")
                g2t = cpool.tile([128, 320], F16, tag="g2t")
                xd = cpool.tile([128, 320], F16, tag="xd")

            # Small-input DMAs are interleaved into the Pool (swdge) queue
            # between the big stream chunks so their transfers land in DMA
            # gaps the ACT stream can afford; slot k fires after chunk k.
            def smalls(hh):
                if "rec" in parts:
                    if hh == 2:
                        nc.gpsimd.dma_start(prt[:], recpred[:])
                    elif hh == 3:
                        nc.gpsimd.dma_start(grt[:], recgt[:])
                    elif hh == 4:
                        nc.gpsimd.dma_start(vrt[:], recvis[:])
                if "diag" in parts:
                    if hh == 5:
                        nc.gpsimd.dma_start(g1t[:], g1c[:])
                        nc.gpsimd.dma_start(g2t[:], g2c[:])
                    elif 9 <= hh <= 12:
                        for m in (2 * (hh - 9), 2 * (hh - 9) + 1):
                            nc.gpsimd.dma_start(
                                xd[:, m * 40 : m * 40 + 40],
                                _ap(corr, m * MATSZ,
                                    [[8 * 1025, 128], [1025, 8], [1, 5]]),
                            )
                if "ident" in parts:
                    if hh == 6:
                        nc.gpsimd.dma_start(psb[:], projbc[:])
                        nc.gpsimd.dma_start(tks[0][:], trk[0])
                    elif hh == 7:
                        nc.gpsimd.dma_start(tks[1][:], trk[1])
                    elif hh == 8:
                        nc.gpsimd.dma_start(pds[0][:], iprd[0])
                        nc.gpsimd.dma_start(pds[1][:], iprd[1])

            # ---------------- main sigmoid + product-tree stream ----------
            # First/last halves are split into quarter chunks to shrink the
            # pipeline fill (first sigmoid starts sooner) and drain (last
            # tree is half as deep).
            def chunkstream(hh, qq, nq):
                """Process rows [qq/nq .. (qq+1)/nq) of half hh."""
                w0 = 4096 // nq
                rc = 4 // nq                       # row-chunks in this chunk
                xt = xpool.tile([128, w0], F16, tag="xt", name="xt")
                q = nc.sync if hh % 2 == 0 else nc.gpsimd
                q.dma_start(
                    xt[:],
                    _ap(corr, 2 + hh * HALF + qq * (HALF // nq),
                        [[1024, 128], [131072, rc], [1, 1024]]),
                )
                if qq == 0:
                    smalls(hh)
                if "main" not in parts:
                    return
                st = spool.tile([128, w0], BF16, tag="st", name="st")
                nc.scalar.activation(st[:], xt[:], AF.Sigmoid, scale=-1.0)
                cur = st
                w = w0
                lvl = 0
                ncols = w0 // KGRP
                while w > 2 * ncols:
                    h = w // 2
                    nxt = tpool.tile([128, h], BF16, tag=f"t{lvl}", name="tl")
                    nc.vector.tensor_tensor(
                        nxt[:], cur[:, 0:h], cur[:, h : 2 * h], OP.mult
                    )
                    cur = nxt
                    w = h
                    lvl += 1
                # last level writes the persistent output tile directly
                h = w // 2
                base = hh * (4096 // KGRP) + qq * ncols
                nc.vector.tensor_tensor(
                    outp[:, base : base + ncols],
                    cur[:, 0:h], cur[:, h : 2 * h], OP.mult,
                )

            def halfstream(hh):
                if hh in (0, NHALF - 1):
                    for qq in range(2):
                        chunkstream(hh, qq, 2)
                else:
                    chunkstream(hh, 0, 1)

            for hh in range(6):
                halfstream(hh)

            # ---------------- reconstruction loss partials ----------------
            if "rec" in parts:
                mf = cpool.tile([128, 64], F32, tag="mf")
                nc.vector.tensor_copy(mf[:], vrt[:])
                dd = cpool.tile([128, 192], F32, tag="dd")
                nc.vector.tensor_tensor(dd[:], prt[:], grt[:], OP.subtract)
                d2 = cpool.tile([128, 192], F32, tag="d2")
                nc.vector.tensor_tensor(d2[:], dd[:], dd[:], OP.mult)
                se3 = cpool.tile([128, 64], F32, tag="se3")
                nc.vector.tensor_reduce(
                    se3[:], d2[:].rearrange("p (q c) -> p q c", c=3), axis=AX.X, op=OP.add
                )
                se3m = cpool.tile([128, 64], F32, tag="se3m")
                nc.vector.tensor_tensor(se3m[:], se3[:], mf[:], OP.mult)
                nc.vector.tensor_reduce(
                    fin[:, C_REC_SE : C_REC_SE + 1], se3m[:], axis=AX.X, op=OP.add
                )
                nc.vector.tensor_reduce(
                    fin[:, C_REC_NUM : C_REC_NUM + 1], mf[:], axis=AX.X, op=OP.add
                )
                # masked min / max of gt per coordinate
                gf = cpool.tile([128, 192], F32, tag="gf")
                nc.vector.tensor_copy(gf[:], grt[:])
                bmn = cpool.tile([128, 192], F32, tag="bmn")
                bmx = cpool.tile([128, 192], F32, tag="bmx")
                nc.vector.memset(bmn[:], 1e30)
                nc.vector.memset(bmx[:], -1e30)
                for c in range(3):
                    nc.vector.copy_predicated(
                        bmn[:, c : 192 : 3], vrt[:], gf[:, c : 192 : 3]
                    )
                    nc.vector.copy_predicated(
                        bmx[:, c : 192 : 3], vrt[:], gf[:, c : 192 : 3]
                    )
                nc.vector.tensor_reduce(
                    fin[:, C_REC_MN : C_REC_MN + 3],
                    bmn[:].rearrange("p (q c) -> p c q", c=3), axis=AX.X, op=OP.min,
                )
                nc.vector.tensor_reduce(
                    fin[:, C_REC_MX : C_REC_MX + 3],
                    bmx[:].rearrange("p (q c) -> p c q", c=3), axis=AX.X, op=OP.max,
                )

            for hh in range(6, 10):
                halfstream(hh)

            # ---------------- identity loss partials (2 vb slots) ----------
            if "ident" in parts:
                for i in range(2):
                    tk = tks[i]
                    pd = pds[i]
                    Xc = pd[:, 0:384:3]
                    Yc = pd[:, 1:384:3]
                    Zc = pd[:, 2:384:3]

                    def cS(col):
                        return psb[:, col : col + 1]

                    base = i * 12

                    def lincomb(row, tag):
                        # P[row,0]*x + P[row,1]*y + P[row,2]*z + P[row,3]
                        t0 = cpool.tile([128, 128], F32, tag=f"{tag}0_{i}")
                        t1 = cpool.tile([128, 128], F32, tag=f"{tag}1_{i}")
                        nc.vector.tensor_scalar(
                            t0[:], Xc, cS(base + row * 4 + 0), cS(base + row * 4 + 3),
                            OP.mult, OP.add,
                        )
                        nc.vector.tensor_scalar(
                            t1[:], Yc, cS(base + row * 4 + 1), None, OP.mult
                        )
                        nc.vector.tensor_tensor(t0[:], t0[:], t1[:], OP.add)
                        nc.vector.tensor_scalar(
                            t1[:], Zc, cS(base + row * 4 + 2), None, OP.mult
                        )
                        nc.vector.tensor_tensor(t0[:], t0[:], t1[:], OP.add)
                        return t0

                    den = lincomb(2, "den")
                    nc.vector.tensor_scalar_add(den[:], den[:], 1e-10)
                    rd = cpool.tile([128, 128], F32, tag=f"rd{i}")
                    nc.vector.reciprocal(rd[:], den[:])
                    nx = lincomb(0, "nx")
                    ny = lincomb(1, "ny")
                    nc.vector.tensor_tensor(nx[:], nx[:], rd[:], OP.mult)
                    nc.vector.tensor_tensor(ny[:], ny[:], rd[:], OP.mult)
                    nc.vector.tensor_tensor(nx[:], nx[:], tk[:, 0:256:2], OP.subtract)
                    nc.vector.tensor_tensor(ny[:], ny[:], tk[:, 1:256:2], OP.subtract)
                    sqx = cpool.tile([128, 128], F32, tag=f"sqx{i}")
                    nc.vector.tensor_tensor(sqx[:], nx[:], nx[:], OP.mult)
                    nc.vector.tensor_reduce(
                        fin[:, C_ID_ERR + 2 * i : C_ID_ERR + 2 * i + 1],
                        sqx[:], axis=AX.X, op=OP.add,
                    )
                    sqy = cpool.tile([128, 128], F32, tag=f"sqy{i}")
                    nc.vector.tensor_tensor(sqy[:], ny[:], ny[:], OP.mult)
                    nc.vector.tensor_reduce(
                        fin[:, C_ID_ERR + 2 * i + 1 : C_ID_ERR + 2 * i + 2],
                        sqy[:], axis=AX.X, op=OP.add,
                    )
                    tkv = tk[:].rearrange("p (q c) -> p c q", c=2)
                    nc.vector.tensor_reduce(
                        fin[:, C_ID_MN + 2 * i : C_ID_MN + 2 * i + 2],
                        tkv, axis=AX.X, op=OP.min,
                    )
                    nc.vector.tensor_reduce(
                        fin[:, C_ID_MX + 2 * i : C_ID_MX + 2 * i + 2],
                        tkv, axis=AX.X, op=OP.max,
                    )

            for hh in range(10, 14):
                halfstream(hh)

            # ---------------- band correction (DVE polynomial) -------------
            if "diag" in parts:
                u = cpool.tile([128, 320], F16, tag="u")
                nc.vector.tensor_tensor(u[:], xd[:], xd[:], OP.mult)
                q1 = cpool.tile([128, 320], F16, tag="q1")
                nc.vector.tensor_scalar(q1[:], u[:], PC2, PC1, OP.mult, OP.add)
                q2 = cpool.tile([128, 320], F16, tag="q2")
                nc.vector.tensor_scalar(q2[:], u[:], PC4, PC3, OP.mult, OP.add)
                u2 = cpool.tile([128, 320], F16, tag="u2")
                nc.vector.tensor_tensor(u2[:], u[:], u[:], OP.mult)
                r = cpool.tile([128, 320], F16, tag="r")
                nc.vector.tensor_tensor(r[:], q2[:], u2[:], OP.mult)
                nc.vector.tensor_tensor(r[:], r[:], q1[:], OP.add)
                P = cpool.tile([128, 320], F16, tag="P")
                nc.vector.tensor_tensor(P[:], r[:], u[:], OP.mult)
                z = cpool.tile([128, 320], F16, tag="z")
                nc.vector.tensor_scalar(z[:], xd[:], 0.5, None, OP.mult)
                nc.vector.tensor_tensor(P[:], P[:], z[:], OP.add)  # sp - c0
                s1 = cpool.tile([128, 320], F16, tag="s1")
                nc.vector.tensor_tensor(s1[:], P[:], g1t[:], OP.mult)
                nc.vector.tensor_reduce(
                    fin[:, C_ATT_C1 : C_ATT_C1 + 1], s1[:], axis=AX.X, op=OP.add
                )
                s2 = cpool.tile([128, 320], F16, tag="s2")
                nc.vector.tensor_tensor(s2[:], xd[:], g2t[:], OP.mult)
                nc.vector.tensor_reduce(
                    fin[:, C_ATT_C2 : C_ATT_C2 + 1], s2[:], axis=AX.X, op=OP.add
                )

            for hh in range(14, NHALF):
                halfstream(hh)

            # ---------------- store partials ----------------
            if "main" in parts:
                pc = PRODC // MAT_PER_CORE
                for m in range(MAT_PER_CORE):
                    nc.sync.dma_start(
                        _ap(prods_d, m * pc, [[PRODC, 128], [1, pc]]),
                        outp[:, m * pc : m * pc + pc],
                    )
            nc.sync.dma_start(out_d[:], fin[:])

    nc.compile()
    return nc


def _host_constants():
    """Banded weights + index tables (data independent)."""
    i_idx = np.arange(128)[:, None] * 8 + np.arange(8)[None, :]        # [128,8]
    d_off = np.arange(5) - 2
    ipd = i_idx[:, :, None] + d_off[None, None, :]                     # [128,8,5]
    valid = (ipd >= 0) & (ipd < N)
    beta = np.array([0.49, 0.7, 1.0, 0.7, 0.49], np.float64)
    b1 = np.where(valid, (2.0 * beta)[None, None, :], 0.0)
    b2 = np.where(valid, (-(beta + 2.0 * beta**2))[None, None, :], 0.0)
    b1 = np.tile(b1.reshape(128, 40), (1, 4))                          # [128,160]
    b2 = np.tile(b2.reshape(128, 40), (1, 4))
    return i_idx, ipd, valid, b1, b2


def kernel(refined_points, gt_points, visibility, projection_matrices,
           tracks_2d, correspondence_matrices):
    global LAST_RESULTS
    refined_points = np.ascontiguousarray(refined_points, np.float32)
    gt_points = np.ascontiguousarray(gt_points, np.float32)
    visibility = np.ascontiguousarray(visibility, np.int32)
    projection_matrices = np.ascontiguousarray(projection_matrices, np.float32)
    tracks_2d = np.ascontiguousarray(tracks_2d, np.float32)
    corr = np.ascontiguousarray(correspondence_matrices, np.float32)

    if "nc" not in _CACHE:
        _CACHE["nc"] = _build_program()
    nc = _CACHE["nc"]

    i_idx, ipd, valid, b1, b2 = _host_constants()
    vis0 = visibility[:, 0, :]                                         # [4,1024]
    visr = np.repeat(vis0[:, i_idx][:, :, :, None], 5, axis=3)         # [4,128,8,5]
    visr = visr.reshape(4, 128, 40).transpose(1, 0, 2).reshape(128, 160)
    visc = np.where(valid[None], vis0[:, np.clip(ipd, 0, N - 1)], 0)   # [4,128,8,5]
    visc = visc.reshape(4, 128, 40).transpose(1, 0, 2).reshape(128, 160)
    pair = np.maximum(visr, visc).astype(np.float64)                   # OR of 0/1
    g1 = np.tile(b1 * pair, (1, 2)).astype(np.float16)                 # [128,320]
    g2 = np.tile(b2 * pair, (1, 2)).astype(np.float16)
    g1sum = float(np.tile(b1 * pair, (1, 2)).sum())                    # for c0 term

    corr16 = corr.reshape(V * V * B, MATSZ).astype(np.float16)
    pred_flat = refined_points.reshape(B * F_FRAMES * N, 3).astype(np.float16)
    gt_flat = gt_points.reshape(B * F_FRAMES * N, 3).astype(np.float16)
    vis_flat = visibility.reshape(B * F_FRAMES * N)
    pvals = projection_matrices.reshape(V * B, 12)
    trk16 = tracks_2d.astype(np.float16)

    in_maps = []
    for c in range(NCORES):
        cp = np.zeros(CORR_LEN, np.float16)
        cp[2:-2] = corr16[c * MAT_PER_CORE : (c + 1) * MAT_PER_CORE].ravel()
        rp = pred_flat[c * REC_PTS : (c + 1) * REC_PTS].reshape(128, 192)
        rg = gt_flat[c * REC_PTS : (c + 1) * REC_PTS].reshape(128, 192)
        rv = vis_flat[c * REC_PTS : (c + 1) * REC_PTS].reshape(128, 64)
        vbs = [2 * c, 2 * c + 1]
        tks = np.stack([trk16[vb // 4, vb % 4].reshape(128, 256) for vb in vbs])
        ipr = np.stack([refined_points[vb % 4].reshape(128, 384) for vb in vbs])
        pb = np.broadcast_to(
            np.concatenate([pvals[vb] for vb in vbs])[None, :], (128, 24)
        )
        in_maps.append({
            "corrpad": cp,
            "recpred": np.ascontiguousarray(rp),
            "recgt": np.ascontiguousarray(rg),
            "recvis": np.ascontiguousarray(rv, np.int32),
            "g1c": g1,
            "g2c": g2,
            "trk": np.ascontiguousarray(tks),
            "iprd": np.ascontiguousarray(ipr),
            "projbc": np.ascontiguousarray(pb, np.float32),
        })

    trace = bool(int(os.environ.get("KERNEL_TRACE", "0")))
    ncr = int(os.environ.get("KERNEL_NCORES", str(NCORES)))
    res = run_bass_kernel_spmd(
        nc, in_maps[:ncr], core_ids=list(range(ncr)), trace=trace,
        stitch_traces=trace,
    )
    LAST_RESULTS = res
    P = np.stack([r["out"] for r in res.results]).astype(np.float64)   # [8,128,24]
    PR = np.stack([np.asarray(r["prods"]) for r in res.results]).astype(np.float64)

    # ---- attention ----
    att_sum = -np.log(PR).sum()
    att_sum += P[:, :, C_ATT_C1].sum() + PC0 * g1sum * ncr
    att_sum += P[:, :, C_ATT_C2].sum()
    att = att_sum / (V * V * B * N * N)

    # ---- reconstruction ----
    se = P[:, :, C_REC_SE].sum()
    num = 3.0 * P[:, :, C_REC_NUM].sum()
    mn = P[:, :, C_REC_MN : C_REC_MN + 3].min(axis=(0, 1))
    mx = P[:, :, C_REC_MX : C_REC_MX + 3].max(axis=(0, 1))
    scale = (mx - mn).max() + 1e-6
    if not num > 0:
        scale = 1.0
    rec = (se / max(num, 1.0)) / scale**2

    # ---- identity ----
    vls = []
    for vb in range(V * B):
        c, i = vb // 2, vb % 2
        ex = P[c, :, C_ID_ERR + 2 * i]
        ey = P[c, :, C_ID_ERR + 2 * i + 1]
        mnx = P[c, :, C_ID_MN + 2 * i]
        mny = P[c, :, C_ID_MN + 2 * i + 1]
        mxx = P[c, :, C_ID_MX + 2 * i]
        mxy = P[c, :, C_ID_MX + 2 * i + 1]
        for f in range(F_FRAMES):
            s = slice(8 * f, 8 * f + 8)
            whx = max(224.0, mxx[s].max() - mnx[s].min() + 1e-6)
            why = max(224.0, mxy[s].max() - mny[s].min() + 1e-6)
            vls.append((ex[s].sum() / whx**2 + ey[s].sum() / why**2) / N)
    ident = float(np.mean(vls))

    total = 1.0 * rec + 1.0 * ident + 0.5 * att
    return (
        np.float32(total), np.float32(rec), np.float32(ident), np.float32(att),
    )

